# revision 1
# baseline (speedup 1.0000x reference)
"""Trainium2 Bass kernel for the DTVIN (dynamic-transition VIN) model.

Self-contained: accepts FULL inputs (batch 128), shards batch over 8
NeuronCores (16 samples/core, pure data parallel), runs one Bass program
per core via run_bass_kernel_spmd, returns full [128, 8, 49, 49] logits.

Per-core program (all fp16 compute, fp32 PSUM accumulation):
  conv1 (K=18 im2col matmul) -> relu -> conv2 (dj-folded im2col, K=450 x
  3 row-shifts) -> relu -> reward/trans conv (pad=2, 51x51, K=450 x 3) ->
  exp on drain -> softmax (tree-sum + reciprocal) -> 30-step value
  iteration on the Vector engine with (action,sample) partition layout ->
  per-pixel MLP (8->150->8) -> logits.
"""
import sys
import types
import numpy as np

for p in ("/opt/trn_rl_repo", "/root/.axon_site/_ro/trn_rl_repo"):
    if p not in sys.path:
        sys.path.append(p)

import concourse.bass as bass
import concourse.tile as tile
from concourse import mybir
from concourse.bass_utils import run_bass_kernel_spmd

FP16 = mybir.dt.float16
FP32 = mybir.dt.float32
AF = mybir.ActivationFunctionType

# geometry
S = 16          # samples per core
HID = 150
A = 8
G = 49          # conv grid
HP = 51         # value-iteration grid
KST = 30        # value-iteration steps
CH, CW = 53, 54          # big canvas (rows, cols) for conv-side tensors
PH, PW = 51, 52          # plane canvas for sT / scan tensors
CFLAT = CH * CW          # 2862
PFLAT = PH * PW          # 2652


def _split_multi_waits(nc):
    """This walrus build accepts at most ONE sync wait per instruction.
    Move extra waits onto preceding same-engine NoOps (queues run in
    order, so nop-waits followed by the inst == waiting on all)."""
    n = 0
    for f in nc.m.functions:
        for bb in f.blocks:
            new_insts = []
            for ins in bb.instructions:
                si = ins.sync_info
                if si is not None and len(si.on_wait) > 1:
                    waits = list(si.on_wait)
                    for w in waits[:-1]:
                        n += 1
                        nop = mybir.InstNoOp(name=f"waitsplit_{n}", ins=[], outs=[])
                        nop.engine = ins.engine
                        nop.sync_info = mybir.SyncInfo(on_wait=[w], on_update=[])
                        new_insts.append(nop)
                    ins.sync_info = mybir.SyncInfo(on_wait=[waits[-1]],
                                                   on_update=list(si.on_update))
                new_insts.append(ins)
            bb.instructions = new_insts


def _part_ap(t, part_start, part_stride, part_num, free_ap, extra_off=0):
    """AP over tile `t` with a strided partition dim (for DMA only)."""
    full = t[:, :] if len(t.shape) == 2 else t[:, :, :] if len(t.shape) == 3 else t[:, :, :, :]
    pstride = full.ap[0][0]  # elements per partition step
    return bass.AP(tensor=full.tensor,
                   offset=full.offset + part_start * pstride + extra_off,
                   ap=[[pstride * part_stride, part_num]] + list(free_ap))


def build_program(debug_taps=False):
    nc = bass.Bass("TRN2", target_bir_lowering=False, debug=False, num_devices=8)

    # ---- DRAM I/O (per core) ----
    grid16 = nc.dram_tensor("grid16", [S, 2, G, G], FP16, kind="ExternalInput")
    w1 = nc.dram_tensor("w1", [18, HID], FP16, kind="ExternalInput")
    b1 = nc.dram_tensor("b1", [HID, 1], FP32, kind="ExternalInput")
    w2 = nc.dram_tensor("w2", [3, 450, HID], FP16, kind="ExternalInput")
    b2 = nc.dram_tensor("b2", [HID, 1], FP32, kind="ExternalInput")
    wrt = nc.dram_tensor("wrt", [3, 450, 97], FP16, kind="ExternalInput")
    wa1 = nc.dram_tensor("wa1", [A, HID], FP16, kind="ExternalInput")
    ba1 = nc.dram_tensor("ba1", [HID, 1], FP32, kind="ExternalInput")
    wa2 = nc.dram_tensor("wa2", [HID, A], FP16, kind="ExternalInput")
    ba2 = nc.dram_tensor("ba2", [A, 1], FP32, kind="ExternalInput")
    out = nc.dram_tensor("o", [S, A, G, G], FP32, kind="ExternalOutput")
    if debug_taps:
        dbg_h1 = nc.dram_tensor("dbg_h1", [128, CH, CW], FP16, kind="ExternalOutput")
        dbg_h = nc.dram_tensor("dbg_h", [128, CH, CW], FP16, kind="ExternalOutput")
        dbg_e = nc.dram_tensor("dbg_e", [72, PH, PW], FP16, kind="ExternalOutput")
        dbg_sT = nc.dram_tensor("dbg_sT", [128, 9, PH, PW], FP16, kind="ExternalOutput")
        dbg_r32 = nc.dram_tensor("dbg_r32", [32, PH, PW], FP16, kind="ExternalOutput")
        dbg_V = nc.dram_tensor("dbg_V", [128, CH, CW], FP16, kind="ExternalOutput")
        dbg_q = nc.dram_tensor("dbg_q", [128, PH, PW], FP16, kind="ExternalOutput")

    KT = [(0, 128), (128, 128), (256, 128), (384, 66)]   # K-tiles of 450
    NCH1 = [(0, 9), (9, 9), (18, 9), (27, 9), (36, 9), (45, 4)]       # 49 rows
    NCH2 = [(0, 9), (9, 9), (18, 9), (27, 9), (36, 9), (45, 6)]       # 51 rows
    MLPN = [(0, 10), (10, 10), (20, 10), (30, 10), (40, 9)]  # row chunks of 49

    with tile.TileContext(nc) as tc:
        import contextlib
        with contextlib.ExitStack() as ctx:
            persist = ctx.enter_context(tc.tile_pool(name="persist", bufs=1))
            psum = ctx.enter_context(tc.tile_pool(name="psum", bufs=3, space="PSUM"))

            # ---------- persistent tiles ----------
            sT = persist.tile([128, 9, PH, PW], FP16, tag="sT")
            V = persist.tile([128, CH, CW], FP16, tag="V")
            reward32 = persist.tile([32, PH, PW], FP16, tag="reward32")
            R_rep = persist.tile([128, PH, PW], FP16, tag="R_rep")
            q_last = persist.tile([128, PH, PW], FP16, tag="q_last")

            # weights in SBUF
            w1t = persist.tile([18, HID], FP16, tag="w1t")
            nc.sync.dma_start(out=w1t, in_=w1[:, :])
            b1A = persist.tile([128, 1], FP32, tag="b1A")
            b1B = persist.tile([22, 1], FP32, tag="b1B")
            nc.sync.dma_start(out=b1A, in_=b1[0:128, :])
            nc.sync.dma_start(out=b1B, in_=b1[128:150, :])
            b2A = persist.tile([128, 1], FP32, tag="b2A")
            b2B = persist.tile([22, 1], FP32, tag="b2B")
            nc.sync.dma_start(out=b2A, in_=b2[0:128, :])
            nc.sync.dma_start(out=b2B, in_=b2[128:150, :])
            w2t = []  # [di][kt] -> [kn, 150]
            wrtt = []
            for di in range(3):
                w2t.append([])
                wrtt.append([])
                for (k0, kn) in KT:
                    t2 = persist.tile([kn, HID], FP16, tag=f"w2_{di}_{k0}", name=f"w2_{di}_{k0}")
                    nc.sync.dma_start(out=t2, in_=w2[di, k0:k0 + kn, :])
                    w2t[di].append(t2)
                    t3 = persist.tile([kn, 97], FP16, tag=f"wrt_{di}_{k0}", name=f"wrt_{di}_{k0}")
                    nc.sync.dma_start(out=t3, in_=wrt[di, k0:k0 + kn, :])
                    wrtt[di].append(t3)
            wa1t = persist.tile([A, HID], FP16, tag="wa1t")
            nc.sync.dma_start(out=wa1t, in_=wa1[:, :])
            wa2A = persist.tile([128, A], FP16, tag="wa2A")
            wa2B = persist.tile([22, A], FP16, tag="wa2B")
            nc.sync.dma_start(out=wa2A, in_=wa2[0:128, :])
            nc.sync.dma_start(out=wa2B, in_=wa2[128:150, :])
            ba1A = persist.tile([128, 1], FP32, tag="ba1A")
            ba1B = persist.tile([22, 1], FP32, tag="ba1B")
            nc.sync.dma_start(out=ba1A, in_=ba1[0:128, :])
            nc.sync.dma_start(out=ba1B, in_=ba1[128:150, :])
            ba2t = persist.tile([A, 1], FP32, tag="ba2t")
            nc.sync.dma_start(out=ba2t, in_=ba2[:, :])

            nc.vector.memset(V.rearrange("p a b -> p (a b)"), 0.0)

            # ---------- conv phase (scoped pool, freed before scan) ----------
            with tc.tile_pool(name="convfix", bufs=1) as cfix:
                # grid canvas [32 = (ci, s), CH, CW], data 49x49 at (1,1)
                Gc = cfix.tile([32, CH, CW], FP16, tag="Gc")
                nc.vector.memset(Gc.rearrange("p a b -> p (a b)"), 0.0)
                for ci in range(2):
                    nc.sync.dma_start(out=Gc[16 * ci:16 * ci + 16, 1:1 + G, 1:1 + G],
                                      in_=grid16[:, ci, :, :])

                # single X1; double-buffered h1/h; X3h shares slot-set X3[1]
                X1 = cfix.tile([18, CH, CW], FP16, tag="X1", name="X1")
                h1A = [cfix.tile([128, CH, CW], FP16, tag=f"h1A{i}", name=f"h1A{i}") for i in range(2)]
                h1B = [cfix.tile([22, CH, CW], FP16, tag=f"h1B{i}", name=f"h1B{i}") for i in range(2)]
                hA = [cfix.tile([128, CH, CW], FP16, tag=f"hA{i}", name=f"hA{i}") for i in range(2)]
                hB = [cfix.tile([22, CH, CW], FP16, tag=f"hB{i}", name=f"hB{i}") for i in range(2)]
                X3 = [[cfix.tile([kn, CH, CW], FP16, tag=f"X3_{k0}_{i}", name=f"X3_{k0}_{i}")
                       for (k0, kn) in KT] for i in range(2)]
                e_s = cfix.tile([72, PH, PW], FP16, tag="e0", name="e0")
                r_s = cfix.tile([1, PH, PW], FP16, tag="r0", name="r0")
                nc.vector.memset(X1.rearrange("p a b -> p (a b)"), 0.0)
                for i in range(2):
                    for t in (h1A[i], h1B[i], hA[i], hB[i]):
                        nc.vector.memset(t.rearrange("p a b -> p (a b)"), 0.0)
                    for t in X3[i]:
                        nc.vector.memset(t.rearrange("p a b -> p (a b)"), 0.0)
                nc.vector.memset(e_s.rearrange("p a b -> p (a b)"), 0.0)
                nc.vector.memset(r_s.rearrange("p a b -> p (a b)"), 0.0)

                qidx = [0]

                def build_x3q(dst_tiles, src_a, src_b):
                    jobs = []
                    for dj in range(3):
                        r = dj * 150
                        for (st, sr0, nr) in ((src_a, 0, 128), (src_b, 0, 22)):
                            left = nr
                            while left:
                                kt = None
                                for idx, (k0, kn) in enumerate(KT):
                                    if k0 <= r < k0 + kn:
                                        kt = idx
                                        break
                                k0, kn = KT[kt]
                                take = min(left, k0 + kn - r)
                                jobs.append((st, sr0 + (nr - left), take, dj, kt, r - k0))
                                r += take
                                left -= take
                    for (st, sr0, nr, dj, kt, dr0) in jobs:
                        soff = dj - 1
                        d0c = max(0, -soff)
                        ln = CFLAT - abs(soff)
                        dt = dst_tiles[kt]
                        dstp = _part_ap(dt, dr0, 1, nr, [[1, ln]], extra_off=d0c)
                        srcp = _part_ap(st, sr0, 1, nr, [[1, ln]],
                                        extra_off=max(0, soff))
                        eng = nc.scalar if qidx[0] % 2 == 0 else nc.sync
                        qidx[0] += 1
                        eng.dma_start(out=dstp, in_=srcp)

                def emit_x1(s):
                    for di in range(3):
                        for dj in range(3):
                            off = di * CW + dj
                            ln = CFLAT - off
                            dst = _part_ap(X1, di * 3 + dj, 9, 2, [[1, ln]])
                            srcp = _part_ap(Gc, s, 16, 2, [[1, ln]], extra_off=off)
                            nc.sync.dma_start(out=dst, in_=srcp)

                def emit_conv1(s):
                    pp = s % 2
                    for (y0, ny) in NCH1:
                        ps = psum.tile([128, 9, 52], FP32, tag="psA", name=f"c1ps{s}_{y0}")
                        nc.tensor.matmul(out=ps[:, 0:ny, :],
                                         lhsT=w1t[:, 0:128],
                                         rhs=X1[:, y0:y0 + ny, 0:52],
                                         start=True, stop=True)
                        nc.scalar.activation(out=h1A[pp][:, 1 + y0:1 + y0 + ny, 2:2 + G],
                                             in_=ps[:, 0:ny, 0:G], func=AF.Relu,
                                             bias=b1A, scale=1.0)
                        ps2 = psum.tile([22, 9, 52], FP32, tag="psB", name=f"c1ps2{s}_{y0}")
                        nc.tensor.matmul(out=ps2[:, 0:ny, :],
                                         lhsT=w1t[:, 128:150],
                                         rhs=X1[:, y0:y0 + ny, 0:52],
                                         start=True, stop=True)
                        nc.scalar.activation(out=h1B[pp][:, 1 + y0:1 + y0 + ny, 2:2 + G],
                                             in_=ps2[:, 0:ny, 0:G], func=AF.Relu,
                                             bias=b1B, scale=1.0)

                def emit_conv2(s):
                    pp = s % 2
                    for (y0, ny) in NCH1:
                        ps = psum.tile([128, 9, 52], FP32, tag="psA", name=f"c2ps{s}_{y0}")
                        nmm = 0
                        for di in range(3):
                            for kt, (k0, kn) in enumerate(KT):
                                rhs = X3[0][kt][:, y0 + di:y0 + di + ny, 2:2 + 52]
                                nc.tensor.matmul(out=ps[:, 0:ny, :],
                                                 lhsT=w2t[di][kt][:, 0:128], rhs=rhs,
                                                 start=(nmm == 0), stop=(nmm == 11))
                                nmm += 1
                        nc.scalar.activation(out=hA[pp][:, 2 + y0:2 + y0 + ny, 2:2 + G],
                                             in_=ps[:, 0:ny, 0:G], func=AF.Relu,
                                             bias=b2A, scale=1.0)
                        psB = psum.tile([22, 9, 52], FP32, tag="psB", name=f"c2psB{s}_{y0}")
                        nmm = 0
                        for di in range(3):
                            for kt, (k0, kn) in enumerate(KT):
                                rhs = X3[0][kt][:, y0 + di:y0 + di + ny, 2:2 + 52]
                                nc.tensor.matmul(out=psB[:, 0:ny, :],
                                                 lhsT=w2t[di][kt][:, 128:150], rhs=rhs,
                                                 start=(nmm == 0), stop=(nmm == 11))
                                nmm += 1
                        nc.scalar.activation(out=hB[pp][:, 2 + y0:2 + y0 + ny, 2:2 + G],
                                             in_=psB[:, 0:ny, 0:G], func=AF.Relu,
                                             bias=b2B, scale=1.0)

                def emit_rt(s):
                    for (y0, ny) in NCH2:
                        ps = psum.tile([97, 9, 51], FP32, tag="psA", name=f"rtps{s}_{y0}")
                        nmm = 0
                        for di in range(3):
                            for kt, (k0, kn) in enumerate(KT):
                                rhs = X3[1][kt][:, y0 + di:y0 + di + ny, 1:1 + 51]
                                nc.tensor.matmul(out=ps[:, 0:ny, :],
                                                 lhsT=wrtt[di][kt], rhs=rhs,
                                                 start=(nmm == 0), stop=(nmm == 11))
                                nmm += 1
                        nc.scalar.activation(out=e_s[:, y0:y0 + ny, 0:51],
                                             in_=ps[0:72, 0:ny, :], func=AF.Exp,
                                             bias=0.0, scale=1.0)
                        nc.scalar.activation(out=r_s[:, y0:y0 + ny, 0:51],
                                             in_=ps[96:97, 0:ny, :], func=AF.Copy,
                                             bias=0.0, scale=1.0)
                    # scatter e into sT: one DMA per action (9 contiguous
                    # src partitions -> one dst partition), spread across both
                    # HWDGE engines so no queue serializes the next sample.
                    for a in range(A):
                        dst = _part_ap(sT, a * 16 + s, 16, 1,
                                       [[PFLAT, 9], [PW, PH], [1, PW]])
                        eng = nc.scalar if a % 2 == 0 else nc.sync
                        eng.dma_start(out=dst, in_=e_s[a * 9:(a + 1) * 9, :, :])
                    nc.sync.dma_start(out=reward32[s:s + 1, :, :], in_=r_s[:, :, :])
                    nc.sync.dma_start(out=reward32[s + 16:s + 17, :, :], in_=r_s[:, :, :])

                # software pipeline: PE stream is conv2(s), conv1(s+1), rt(s)
                # so the X3h(s)/X3(s+1) builds hide under PE work.
                emit_x1(0)
                emit_conv1(0)
                build_x3q(X3[0], h1A[0], h1B[0])
                for s in range(S):
                    emit_conv2(s)
                    build_x3q(X3[1], hA[s % 2], hB[s % 2])
                    if s + 1 < S:
                        emit_x1(s + 1)
                        emit_conv1(s + 1)
                    emit_rt(s)
                    if s + 1 < S:
                        build_x3q(X3[0], h1A[(s + 1) % 2], h1B[(s + 1) % 2])
                    if debug_taps and s == 0:
                        nc.sync.dma_start(out=dbg_h1[:, :, :], in_=h1A[0][:, :, :])
                        nc.sync.dma_start(out=dbg_h[:, :, :], in_=hA[0][:, :, :])
                        nc.sync.dma_start(out=dbg_e[:, :, :], in_=e_s[:, :, :])

            # ---------- softmax over k (in (a,s) layout) ----------
            with tc.tile_pool(name="smx", bufs=2) as smx:
                d0 = smx.tile([128, PFLAT], FP16, tag="sxa")
                d1 = smx.tile([128, PFLAT], FP16, tag="sxb")
                d2 = smx.tile([128, PFLAT], FP16, tag="sxc")
                sTf = sT.rearrange("p k a b -> p k (a b)")
                nc.vector.tensor_add(out=d0, in0=sTf[:, 0, :], in1=sTf[:, 1, :])
                nc.vector.tensor_add(out=d1, in0=sTf[:, 2, :], in1=sTf[:, 3, :])
                nc.vector.tensor_add(out=d0, in0=d0, in1=d1)
                nc.vector.tensor_add(out=d1, in0=sTf[:, 4, :], in1=sTf[:, 5, :])
                nc.vector.tensor_add(out=d2, in0=sTf[:, 6, :], in1=sTf[:, 7, :])
                nc.vector.tensor_add(out=d1, in0=d1, in1=d2)
                nc.vector.tensor_add(out=d0, in0=d0, in1=d1)
                nc.vector.tensor_add(out=d0, in0=d0, in1=sTf[:, 8, :])
                rec = smx.tile([128, PFLAT], FP16, tag="sxr")
                with nc.allow_low_precision("fp16 softmax denom"):
                    nc.vector.reciprocal(out=rec, in_=d0)
                # halo col 51 of each plane: denom==0 -> rec==inf -> 0*inf=NaN.
                # Zero rec there so sT's halo column stays exactly 0.
                rec_v = rec.rearrange("p (y x) -> p y x", x=PW)
                nc.vector.memset(rec_v[:, :, PW - 1:PW], 0.0)
                for k in range(9):
                    nc.vector.tensor_mul(out=sTf[:, k, :], in0=sTf[:, k, :], in1=rec)

            if debug_taps:
                nc.sync.dma_start(out=dbg_sT[:, :, :, :], in_=sT[:, :, :, :])
                nc.sync.dma_start(out=dbg_r32[:, :, :], in_=reward32[:, :, :])

            # R_rep: replicate reward32 into all 128 partitions
            r32f = reward32.rearrange("p a b -> p (a b)")
            Rf = R_rep.rearrange("p a b -> p (a b)")
            nc.vector.tensor_copy(out=Rf[0:32, :], in_=r32f)
            nc.vector.tensor_copy(out=Rf[32:64, :], in_=Rf[0:32, :])
            nc.vector.tensor_copy(out=Rf[64:128, :], in_=Rf[0:64, :])

            # ---------- value-iteration scan ----------
            with tc.tile_pool(name="scan", bufs=12) as scan, \
                 tc.tile_pool(name="scantail", bufs=2) as tail:
                swap_mask = list(range(16, 32)) + list(range(0, 16))
                Vf = V.rearrange("p a b -> p (a b)")
                for t in range(KST):
                    m = []
                    for k in range(9):
                        di, dj = k // 3, k % 3
                        mk = scan.tile([128, PH, PW], FP16, tag="m")
                        nc.vector.tensor_mul(out=mk,
                                             in0=sT[:, k, :, :],
                                             in1=V[:, di:di + PH, dj:dj + PW])
                        m.append(mk.rearrange("p a b -> p (a b)"))
                    # pairwise tree (in-place into earlier tiles)
                    nc.vector.tensor_add(out=m[0], in0=m[0], in1=m[1])
                    nc.vector.tensor_add(out=m[2], in0=m[2], in1=m[3])
                    nc.vector.tensor_add(out=m[4], in0=m[4], in1=m[5])
                    nc.vector.tensor_add(out=m[6], in0=m[6], in1=m[7])
                    nc.vector.tensor_add(out=m[0], in0=m[0], in1=m[2])
                    nc.vector.tensor_add(out=m[4], in0=m[4], in1=m[6])
                    nc.vector.tensor_add(out=m[0], in0=m[0], in1=m[4])
                    nc.vector.tensor_add(out=m[0], in0=m[0], in1=m[8])
                    Sf = m[0]
                    if t == KST - 1:
                        nc.vector.tensor_add(out=q_last.rearrange("p a b -> p (a b)"),
                                             in0=Sf, in1=Rf)
                        break
                    cp1 = tail.tile([64, PFLAT], FP16, tag="cp1")
                    nc.vector.tensor_copy(out=cp1, in_=Sf[64:128, :])
                    nc.vector.tensor_max(out=cp1, in0=Sf[0:64, :], in1=cp1)
                    cp2 = tail.tile([32, PFLAT], FP16, tag="cp2")
                    nc.vector.tensor_copy(out=cp2, in_=cp1[32:64, :])
                    nc.vector.tensor_max(out=cp2, in0=cp1[0:32, :], in1=cp2)
                    sh = tail.tile([32, PFLAT], FP16, tag="sh")
                    nc.vector.stream_shuffle(out=sh, in_=cp2, mask=swap_mask)
                    nc.vector.tensor_max(out=cp2, in0=cp2, in1=sh)
                    vn = tail.tile([32, PH, PW], FP16, tag="vn")
                    nc.vector.tensor_add(out=vn.rearrange("p a b -> p (a b)"),
                                         in0=cp2, in1=r32f)
                    # scatter + replicate into V
                    nc.vector.tensor_copy(out=V[0:32, 1:1 + PH, 1:1 + PW], in_=vn)
                    nc.vector.tensor_copy(out=Vf[32:64, :], in_=Vf[0:32, :])
                    nc.vector.tensor_copy(out=Vf[64:128, :], in_=Vf[0:64, :])

            if debug_taps:
                nc.sync.dma_start(out=dbg_V[:, :, :], in_=V[:, :, :])
                nc.sync.dma_start(out=dbg_q[:, :, :], in_=q_last[:, :, :])

            # ---------- per-pixel MLP ----------
            with tc.tile_pool(name="mlp", bufs=2) as mlp:
                for s in range(S):
                    # flat gather of this sample's 8 action planes (8 descriptors)
                    qin = mlp.tile([A, PH, PW], FP16, tag="qin")
                    nc.sync.dma_start(
                        out=qin[:, :, :],
                        in_=_part_ap(q_last, s, 16, A, [[1, PFLAT]]))
                    midA = mlp.tile([128, G, G], FP16, tag="midA")
                    midB = mlp.tile([22, G, G], FP16, tag="midB")
                    for (y0, ny) in MLPN:
                        rhs = qin[:, y0:y0 + ny, 0:G]
                        p1 = psum.tile([128, 10, G], FP32, tag="psA")
                        nc.tensor.matmul(out=p1[:, 0:ny, :],
                                         lhsT=wa1t[:, 0:128], rhs=rhs,
                                         start=True, stop=True)
                        nc.scalar.activation(out=midA[:, y0:y0 + ny, :],
                                             in_=p1[:, 0:ny, :], func=AF.Relu,
                                             bias=ba1A, scale=1.0)
                        p2 = psum.tile([22, 10, G], FP32, tag="psB")
                        nc.tensor.matmul(out=p2[:, 0:ny, :],
                                         lhsT=wa1t[:, 128:150], rhs=rhs,
                                         start=True, stop=True)
                        nc.scalar.activation(out=midB[:, y0:y0 + ny, :],
                                             in_=p2[:, 0:ny, :], func=AF.Relu,
                                             bias=ba1B, scale=1.0)
                    ost = mlp.tile([A, G, G], FP32, tag="ost")
                    for (y0, ny) in MLPN:
                        p3 = psum.tile([A, 10, G], FP32, tag="psB")
                        nc.tensor.matmul(out=p3[:, 0:ny, :], lhsT=wa2A,
                                         rhs=midA[:, y0:y0 + ny, :],
                                         start=True, stop=False)
                        nc.tensor.matmul(out=p3[:, 0:ny, :], lhsT=wa2B,
                                         rhs=midB[:, y0:y0 + ny, :],
                                         start=False, stop=True)
                        nc.scalar.activation(out=ost[:, y0:y0 + ny, :],
                                             in_=p3[:, 0:ny, :], func=AF.Identity,
                                             bias=ba2t, scale=1.0)
                    nc.scalar.dma_start(
                        out=out[s, :, :, :],
                        in_=ost[:, :, :])

    _split_multi_waits(nc)
    return nc


def _prep_weights(inputs):
    f32 = lambda x: np.asarray(x, dtype=np.float32)
    h1_w = f32(inputs["h1_w"]); h1_b = f32(inputs["h1_b"])
    h2_w = f32(inputs["h2_w"]); h2_b = f32(inputs["h2_b"])
    r_w = f32(inputs["r_w"]); t_w = f32(inputs["t_w"])
    a1_w = f32(inputs["a1_w"]); a1_b = f32(inputs["a1_b"])
    a2_w = f32(inputs["a2_w"]); a2_b = f32(inputs["a2_b"])

    # conv1: k = ci*9 + di*3 + dj
    w1 = h1_w.transpose(1, 2, 3, 0).reshape(18, HID).astype(np.float16)
    # conv2 / rt: r = dj*150 + ci, one weight set per di
    w2 = np.ascontiguousarray(
        h2_w.transpose(2, 3, 1, 0), dtype=np.float32)  # [di, dj, ci, co]
    w2 = w2.transpose(0, 1, 2, 3).reshape(3, 450, HID).astype(np.float16)
    wrt_full = np.zeros((3, 450, 97), np.float32)
    tw = t_w.transpose(2, 3, 1, 0).reshape(3, 450, 72)   # [di, (dj,ci), m]
    rw = r_w.transpose(2, 3, 1, 0).reshape(3, 450, 1)
    wrt_full[:, :, 0:72] = tw
    wrt_full[:, :, 96:97] = rw
    wrt = wrt_full.astype(np.float16)
    return {
        "w1": w1, "b1": h1_b.reshape(HID, 1),
        "w2": w2, "b2": h2_b.reshape(HID, 1),
        "wrt": wrt,
        "wa1": a1_w.T.astype(np.float16).copy(),      # [8, 150]
        "ba1": a1_b.reshape(HID, 1),
        "wa2": a2_w.T.astype(np.float16).copy(),      # [150, 8]
        "ba2": a2_b.reshape(A, 1),
    }


_CACHE = {}


def _get_program():
    if "nc" not in _CACHE:
        _CACHE["nc"] = build_program()
    return _CACHE["nc"]


def kernel(**inputs):
    nc = _get_program()
    grid = np.asarray(inputs["grid"], dtype=np.float32)
    wts = _prep_weights(inputs)
    in_maps = []
    for c in range(8):
        m = {"grid16": grid[c * S:(c + 1) * S].astype(np.float16)}
        m.update(wts)
        in_maps.append(m)
    res = run_bass_kernel_spmd(nc, in_maps, core_ids=list(range(8)))
    outp = np.concatenate([res.results[c]["o"] for c in range(8)], axis=0)
    return outp.astype(np.float32)


def run_traced(inputs, tmpdir):
    """Like kernel() but with NTFF profiling; returns (output, exec_time_ns)."""
    import ctypes, contextlib
    sys.path.insert(0, "/root/.axon_site/trn_agent_boot")
    import trn_boot
    hook = trn_boot._ntff_profile_via_ctypes("/opt/axon/libaxon_pjrt.so")
    mod = types.ModuleType("antenv.axon_hooks")
    mod.get_axon_ntff_profile_hook = lambda: hook
    sys.modules["antenv.axon_hooks"] = mod

    nc = _get_program()
    grid = np.asarray(inputs["grid"], dtype=np.float32)
    wts = _prep_weights(inputs)
    in_maps = []
    for c in range(8):
        m = {"grid16": grid[c * S:(c + 1) * S].astype(np.float16)}
        m.update(wts)
        in_maps.append(m)
    res = run_bass_kernel_spmd(nc, in_maps, core_ids=list(range(8)),
                               trace=True, tmpdir=tmpdir)
    outp = np.concatenate([res.results[c]["o"] for c in range(8)], axis=0)
    return outp.astype(np.float32), res.exec_time_ns

